# revision 14
# baseline (speedup 1.0000x reference)
"""DeepLSTM Trainium2 kernel (8 NeuronCores, data-parallel over batch).

Model (see reference): per timestep t:
    F = sigmoid(MLP_Fh(h) + MLP_Fx(x_t)); I,O likewise; C = tanh(MLP_Ch(h) + MLP_Cx(x_t))
    c = F*c + I*C ; h = O*tanh(c)
Each MLP is 128 -> 256 -> 256 -> 128 with ReLU between layers.

Strategy per core (batch shard of 32 rows):
  * The 4 x-MLPs do not depend on the recurrence: precompute them for all
    (t, b) as large feature-major matmuls (N=512 moving dim), results ("Gx",
    fp16, L3-bias folded in) stay resident in SBUF.
  * The 4 h-MLPs run in the sequential 512-step loop, fully in feature-major
    layout [feat=128 partitions, batch=32 free] so no transposes are needed
    anywhere.  Weights stationary fp16 (FWL), activations moving.
  * fp32 cell state + PSUM accumulation; fp16 weights/activations/Gx.

Host-side prep (not in HW exec time): shard batch, transpose x to
[feat, t, b] fp16, pack weights into one [128, 8192] fp16 tensor of
128-row k-slices, broadcast/fold biases.
"""

import os

os.environ.setdefault("MYCRO_LOCAL_CACHE", "1")

from contextlib import ExitStack

import numpy as np

import concourse.bass as bass
import concourse.mybir as mybir
import concourse.tile as tile
from concourse import bacc
from concourse.bass import ds
from concourse.bass_utils import run_bass_kernel_spmd

P = 128
HIDDEN = 128
INPUT = 128
BATCH = 256
SEQ = 512
NCORES = 8
BC = BATCH // NCORES  # 32 batch rows per core
GATE_NAMES = ["F", "I", "O", "C_hat"]  # F,I,O sigmoid; C_hat tanh
FP16 = mybir.dt.float16
F32 = mybir.dt.float32
AF = mybir.ActivationFunctionType
ALU = mybir.AluOpType

# weight pack layout: 8 MLPs (F_h,I_h,O_h,C_hat_h,F_x,I_x,O_x,C_hat_x),
# 1024 columns each:
#   [0,256)    W1 [128in, 256out]
#   [256,512)  W2 rows 0:128   (k-slice 0) [128, 256]
#   [512,768)  W2 rows 128:256 (k-slice 1) [128, 256]
#   [768,896)  W3 rows 0:128   [128, 128]
#   [896,1024) W3 rows 128:256 [128, 128]
MLP_COLS = 1024
WPACK_COLS = 8 * MLP_COLS


def build_nc(T=SEQ, unroll=16, compile=True):
    nc = bacc.Bacc()
    xT_d = nc.dram_tensor("xT", [P, T, BC], FP16, kind="ExternalInput")
    wpack_d = nc.dram_tensor("wpack", [P, WPACK_COLS], FP16, kind="ExternalInput")
    bb1_d = nc.dram_tensor("bb1", [P, 8, BC], F32, kind="ExternalInput")
    bb2_d = nc.dram_tensor("bb2", [P, 8, BC], F32, kind="ExternalInput")
    bx1_d = nc.dram_tensor("bx1", [P, 8], F32, kind="ExternalInput")
    bx2_d = nc.dram_tensor("bx2", [P, 8], F32, kind="ExternalInput")
    bx3_d = nc.dram_tensor("bx3", [P, 4], F32, kind="ExternalInput")
    out_d = nc.dram_tensor("hT_out", [P, BC], F32, kind="ExternalOutput")

    with tile.TileContext(nc) as tc, ExitStack() as ctx:
        const = ctx.enter_context(tc.tile_pool(name="const", bufs=1))
        wsb = const.tile([P, WPACK_COLS], FP16)
        nc.sync.dma_start(out=wsb, in_=wpack_d[:, :])
        bb1 = const.tile([P, 8, BC], F32)
        nc.sync.dma_start(out=bb1, in_=bb1_d[:, :, :])
        bb2 = const.tile([P, 8, BC], F32)
        nc.sync.dma_start(out=bb2, in_=bb2_d[:, :, :])
        bx1 = const.tile([P, 8], F32)
        nc.sync.dma_start(out=bx1, in_=bx1_d[:, :])
        bx2 = const.tile([P, 8], F32)
        nc.sync.dma_start(out=bx2, in_=bx2_d[:, :])
        bx3 = const.tile([P, 4], F32)
        nc.sync.dma_start(out=bx3, in_=bx3_d[:, :])
        xsb = const.tile([P, T, BC], FP16)
        nc.sync.dma_start(out=xsb, in_=xT_d[:, :, :])
        # Gx: x-MLP outputs + combined L3 bias, [feat, gate, t*BC]
        gx = const.tile([P, 4, T * BC], FP16)

        # ---------------- x-path: 4 x-MLPs over all (t,b), chunks of 512 cols
        CH = 512  # moving-dim chunk (max fp32-psum free dim)
        TCH = CH // BC  # timesteps per chunk
        nchunks = (T * BC) // CH
        with (
            tc.tile_pool(name="xps", bufs=6, space="PSUM") as xps,
            tc.tile_pool(name="xact", bufs=3) as xact,
        ):
            for c in range(nchunks):
                rhs_x = xsb[:, c * TCH : (c + 1) * TCH, :]
                for g in range(4):
                    base = (4 + g) * MLP_COLS
                    y1 = xact.tile([P, 2, CH], FP16, tag="y1")
                    for m in range(2):
                        ps = xps.tile([P, CH], F32, tag="xps")
                        nc.tensor.matmul(
                            ps,
                            wsb[:, base + m * P : base + (m + 1) * P],
                            rhs_x,
                            start=True,
                            stop=True,
                        )
                        s = g * 2 + m
                        # bias + relu fused on ScalarE (bias is per-partition here)
                        nc.scalar.activation(
                            y1[:, m, :], ps, AF.Relu, bias=bx1[:, s : s + 1]
                        )
                    y2 = xact.tile([P, 2, CH], FP16, tag="y2")
                    for m in range(2):
                        ps = xps.tile([P, CH], F32, tag="xps")
                        for k in range(2):
                            nc.tensor.matmul(
                                ps,
                                wsb[
                                    :,
                                    base + 256 + k * 256 + m * P : base
                                    + 256
                                    + k * 256
                                    + (m + 1) * P,
                                ],
                                y1[:, k, :],
                                start=(k == 0),
                                stop=(k == 1),
                            )
                        s = g * 2 + m
                        # bias + relu fused on VectorE (balance engines)
                        nc.vector.tensor_scalar(
                            out=y2[:, m, :],
                            in0=ps,
                            scalar1=bx2[:, s : s + 1],
                            scalar2=0.0,
                            op0=ALU.add,
                            op1=ALU.max,
                        )
                    ps3 = xps.tile([P, CH], F32, tag="xps")
                    for k in range(2):
                        nc.tensor.matmul(
                            ps3,
                            wsb[:, base + 768 + k * P : base + 768 + (k + 1) * P],
                            y2[:, k, :],
                            start=(k == 0),
                            stop=(k == 1),
                        )
                    # add combined bias (bx3 + bh3), store fp16 into Gx
                    nc.scalar.activation(
                        gx[:, g, c * CH : (c + 1) * CH],
                        ps3,
                        AF.Identity,
                        bias=bx3[:, g : g + 1],
                    )

        # ---------------- recurrence
        state = ctx.enter_context(tc.tile_pool(name="state", bufs=1))
        hT = state.tile([P, BC], FP16)
        nc.vector.memset(hT, 0.0)
        cT = state.tile([P, BC], F32)
        nc.vector.memset(cT, 0.0)

        with (
            tc.tile_pool(name="hps", bufs=2, space="PSUM") as hps,
            tc.tile_pool(name="hact", bufs=3) as hact,
        ):

            def step(t_off, j_off=0, gx_src=None):
                # gx_src: staged per-body Gx window (static slices) or gx
                # itself (static python-unrolled mode, t_off is an int).
                if gx_src is None:
                    gx_src = gx
                ps1 = hps.tile([P, 8, BC], F32, tag="ps1")
                for g in range(4):
                    base = g * MLP_COLS
                    for m in range(2):
                        nc.tensor.matmul(
                            ps1[:, g * 2 + m, :],
                            wsb[:, base + m * P : base + (m + 1) * P],
                            hT,
                            start=True,
                            stop=True,
                        )
                t1 = hact.tile([P, 8, BC], FP16, tag="t1")
                nc.vector.tensor_tensor(out=t1, in0=ps1, in1=bb1, op=ALU.add)
                y1 = hact.tile([P, 8, BC], FP16, tag="hy1")
                nc.vector.tensor_scalar_max(y1, t1, 0.0)

                ps2 = hps.tile([P, 8, BC], F32, tag="ps2")
                for g in range(4):
                    base = g * MLP_COLS
                    for m in range(2):
                        for k in range(2):
                            nc.tensor.matmul(
                                ps2[:, g * 2 + m, :],
                                wsb[
                                    :,
                                    base + 256 + k * 256 + m * P : base
                                    + 256
                                    + k * 256
                                    + (m + 1) * P,
                                ],
                                y1[:, g * 2 + k, :],
                                start=(k == 0),
                                stop=(k == 1),
                            )
                t2 = hact.tile([P, 8, BC], FP16, tag="t2")
                nc.vector.tensor_tensor(out=t2, in0=ps2, in1=bb2, op=ALU.add)
                y2 = hact.tile([P, 8, BC], FP16, tag="hy2")
                nc.vector.tensor_scalar_max(y2, t2, 0.0)

                ps3 = hps.tile([P, 4, BC], F32, tag="ps3")
                for g in range(4):
                    base = g * MLP_COLS
                    for k in range(2):
                        nc.tensor.matmul(
                            ps3[:, g, :],
                            wsb[:, base + 768 + k * P : base + 768 + (k + 1) * P],
                            y2[:, g * 2 + k, :],
                            start=(k == 0),
                            stop=(k == 1),
                        )
                pre = hact.tile([P, 4, BC], FP16, tag="pre")
                if gx_src is gx:
                    gx_in = gx[:, :, t_off : t_off + BC]
                else:
                    gx_in = gx_src[:, :, j_off : j_off + BC]
                nc.vector.tensor_tensor(out=pre, in0=ps3, in1=gx_in, op=ALU.add)
                gsig = hact.tile([P, 3, BC], FP16, tag="gsig")
                nc.scalar.activation(gsig, pre[:, 0:3, :], AF.Sigmoid)
                gch = hact.tile([P, BC], FP16, tag="gch")
                nc.scalar.activation(gch, pre[:, 3, :], AF.Tanh)
                fc = hact.tile([P, BC], F32, tag="fc")
                nc.vector.tensor_tensor(out=fc, in0=gsig[:, 0, :], in1=cT, op=ALU.mult)
                ic = hact.tile([P, BC], F32, tag="ic")
                nc.vector.tensor_tensor(out=ic, in0=gsig[:, 1, :], in1=gch, op=ALU.mult)
                nc.vector.tensor_tensor(out=cT, in0=fc, in1=ic, op=ALU.add)
                th = hact.tile([P, BC], FP16, tag="th")
                nc.scalar.activation(th, cT, AF.Tanh)
                nc.vector.tensor_tensor(out=hT, in0=gsig[:, 2, :], in1=th, op=ALU.mult)

            if unroll >= T:
                for t in range(T):
                    step(t * BC)
            else:
                assert T % unroll == 0
                with tc.For_i(
                    0, T, unroll, hint_engines=(mybir.EngineType.PE,)
                ) as iv:
                    t0 = nc.vector.snap(
                        iv * BC, min_val=0, max_val=(T - unroll) * BC
                    )
                    # one dynamic-AP instruction per body: stage the Gx
                    # window for all `unroll` steps (static slices after)
                    stage = hact.tile([P, 4, unroll * BC], FP16, tag="gxstage")
                    nc.vector.tensor_copy(stage, gx[:, :, ds(t0, unroll * BC)])
                    for j in range(unroll):
                        step(t0, j_off=j * BC, gx_src=stage)

        outsb = state.tile([P, BC], F32)
        nc.vector.tensor_copy(outsb, hT)
        nc.sync.dma_start(out=out_d[:, :], in_=outsb)

    if compile:
        nc.compile()
    return nc


def _np(a):
    return np.asarray(a, dtype=np.float32)


def _prep_shared(params):
    """Pack weights/biases into the kernel's DRAM layouts (replicated)."""
    order = ["F_h", "I_h", "O_h", "C_hat_h", "F_x", "I_x", "O_x", "C_hat_x"]
    wpack = np.zeros((P, WPACK_COLS), dtype=np.float16)
    for i, name in enumerate(order):
        (W1, b1), (W2, b2), (W3, b3) = [( _np(w), _np(b)) for w, b in params[name]]
        base = i * MLP_COLS
        wpack[:, base : base + 256] = W1.astype(np.float16)
        wpack[:, base + 256 : base + 512] = W2[0:128, :].astype(np.float16)
        wpack[:, base + 512 : base + 768] = W2[128:256, :].astype(np.float16)
        wpack[:, base + 768 : base + 896] = W3[0:128, :].astype(np.float16)
        wpack[:, base + 896 : base + 1024] = W3[128:256, :].astype(np.float16)

    bb1 = np.zeros((P, 8, BC), dtype=np.float32)
    bb2 = np.zeros((P, 8, BC), dtype=np.float32)
    bx1 = np.zeros((P, 8), dtype=np.float32)
    bx2 = np.zeros((P, 8), dtype=np.float32)
    bx3 = np.zeros((P, 4), dtype=np.float32)
    for g, gate in enumerate(GATE_NAMES):
        bh = [_np(b) for _, b in params[gate + "_h"]]
        bx = [_np(b) for _, b in params[gate + "_x"]]
        for m in range(2):
            bb1[:, g * 2 + m, :] = bh[0][m * P : (m + 1) * P, None]
            bb2[:, g * 2 + m, :] = bh[1][m * P : (m + 1) * P, None]
            bx1[:, g * 2 + m] = bx[0][m * P : (m + 1) * P]
            bx2[:, g * 2 + m] = bx[1][m * P : (m + 1) * P]
        bx3[:, g] = bx[2] + bh[2]
    return dict(wpack=wpack, bb1=bb1, bb2=bb2, bx1=bx1, bx2=bx2, bx3=bx3)


def _run(x, params, T=SEQ, unroll=16, trace=False):
    x = _np(x)
    B = x.shape[0]
    assert B == BATCH and x.shape[2] == INPUT
    shared = _prep_shared(params)
    in_maps = []
    for core in range(NCORES):
        xc = x[core * BC : (core + 1) * BC, :T, :]  # [BC, T, 128]
        xT = np.ascontiguousarray(xc.transpose(2, 1, 0)).astype(np.float16)
        m = dict(shared)
        m["xT"] = xT
        in_maps.append(m)
    nc = build_nc(T=T, unroll=unroll)
    res = run_bass_kernel_spmd(nc, in_maps, core_ids=list(range(NCORES)), trace=trace)
    out = np.zeros((BATCH, HIDDEN), dtype=np.float32)
    for core in range(NCORES):
        out[core * BC : (core + 1) * BC, :] = res.results[core]["hT_out"].T
    return out, res


def kernel(x, params):
    out, _ = _run(x, params)
    return out


# revision 35
# speedup vs baseline: 1.0359x; 1.0359x over previous
"""DeepLSTM Trainium2 kernel (8 NeuronCores, data-parallel over batch).

Model (see reference): per timestep t:
    F = sigmoid(MLP_Fh(h) + MLP_Fx(x_t)); I,O likewise; C = tanh(MLP_Ch(h) + MLP_Cx(x_t))
    c = F*c + I*C ; h = O*tanh(c)
Each MLP is 128 -> 256 -> 256 -> 128 with ReLU between layers.

Strategy per core (batch shard of 32 rows):
  * The 4 x-MLPs do not depend on the recurrence: precompute them for all
    (t, b) as large feature-major matmuls (N=512 moving dim), results ("Gx",
    fp16, L3-bias folded in) stay resident in SBUF.
  * The 4 h-MLPs run in the sequential 512-step loop, fully in feature-major
    layout [feat=128 partitions, batch=32 free] so no transposes are needed
    anywhere.  Weights stationary fp16 (FWL), activations moving.
  * fp32 cell state + PSUM accumulation; fp16 weights/activations/Gx.

Recurrence runs as two interleaved half-batch streams (16 rows each)
whose independent dependency chains overlap across engines; L1/L2 biases
are injected into PSUM by a rank-8 one-hot matmul; ReLUs read PSUM
directly on VectorE; gate mults run on the otherwise-idle GpSimd engine.

Host-side prep (not in HW exec time): shard batch, transpose x to
[feat, t, b] fp16, pack weights into [128, 4096] fp16 tensors of
128-row k-slices, fold x-path L3 bias + h-path L3 bias together.
"""

import os

os.environ.setdefault("MYCRO_LOCAL_CACHE", "1")

from contextlib import ExitStack

import numpy as np

import concourse.bass as bass
import concourse.mybir as mybir
import concourse.tile as tile
from concourse import bacc
from concourse.bass import ds
from concourse.bass_utils import run_bass_kernel_spmd

P = 128
HIDDEN = 128
INPUT = 128
BATCH = 256
SEQ = 512
NCORES = 8
BC = BATCH // NCORES  # 32 batch rows per core
GATE_NAMES = ["F", "I", "O", "C_hat"]  # F,I,O sigmoid; C_hat tanh
FP16 = mybir.dt.float16
F32 = mybir.dt.float32
AF = mybir.ActivationFunctionType
ALU = mybir.AluOpType

# weight pack layout: 8 MLPs (F_h,I_h,O_h,C_hat_h,F_x,I_x,O_x,C_hat_x),
# 1024 columns each:
#   [0,256)    W1 [128in, 256out]
#   [256,512)  W2 rows 0:128   (k-slice 0) [128, 256]
#   [512,768)  W2 rows 128:256 (k-slice 1) [128, 256]
#   [768,896)  W3 rows 0:128   [128, 128]
#   [896,1024) W3 rows 128:256 [128, 128]
MLP_COLS = 1024
WPACK_COLS = 8 * MLP_COLS


def build_nc(T=SEQ, unroll=32, compile=True):
    nc = bacc.Bacc()
    FP8 = mybir.dt.float8e4
    xT_d = nc.dram_tensor("xT", [P, T, BC], FP16, kind="ExternalInput")
    # h-MLP weights in fp8e4m3: FWL loads 4 cols/cycle (vs 2 for fp16),
    # halving the LDWEIGHTS stream that dominates the recurrence PE time.
    whp_d = nc.dram_tensor("whp", [P, 4 * MLP_COLS], FP16, kind="ExternalInput")
    wxp_d = nc.dram_tensor("wxp", [P, 4 * MLP_COLS], FP16, kind="ExternalInput")
    bx1_d = nc.dram_tensor("bx1", [P, 8], F32, kind="ExternalInput")
    bx2_d = nc.dram_tensor("bx2", [P, 8], F32, kind="ExternalInput")
    bx3_d = nc.dram_tensor("bx3", [P, 4], F32, kind="ExternalInput")
    # rank-8 bias injection: bpk{1,2}[s, m] = layer bias of slot s (=g*2+m),
    # e8[k, (s, b)] = (k == s); psum += bpk.T @ e8 broadcasts each slot's
    # per-feature bias across its batch columns in one matmul.
    bpk1_d = nc.dram_tensor("bpk1", [8, P], FP16, kind="ExternalInput")
    bpk2_d = nc.dram_tensor("bpk2", [8, P], FP16, kind="ExternalInput")
    e8_d = nc.dram_tensor("e8", [8, 2, 8, BC // 2], FP16, kind="ExternalInput")
    out_d = nc.dram_tensor("hT_out", [P, BC], F32, kind="ExternalOutput")

    with tile.TileContext(nc) as tc, ExitStack() as ctx:
        const = ctx.enter_context(tc.tile_pool(name="const", bufs=1))
        whs = const.tile([P, 4 * MLP_COLS], FP16)
        nc.sync.dma_start(out=whs, in_=whp_d[:, :])
        wxs = const.tile([P, 4 * MLP_COLS], FP16)
        nc.sync.dma_start(out=wxs, in_=wxp_d[:, :])
        bx1 = const.tile([P, 8], F32)
        nc.sync.dma_start(out=bx1, in_=bx1_d[:, :])
        bx2 = const.tile([P, 8], F32)
        nc.sync.dma_start(out=bx2, in_=bx2_d[:, :])
        bx3 = const.tile([P, 4], F32)
        nc.sync.dma_start(out=bx3, in_=bx3_d[:, :])
        bpk1 = const.tile([8, P], FP16)
        nc.sync.dma_start(out=bpk1, in_=bpk1_d[:, :])
        bpk2 = const.tile([8, P], FP16)
        nc.sync.dma_start(out=bpk2, in_=bpk2_d[:, :])
        e8 = const.tile([8, 2, 8, BC // 2], FP16)
        nc.sync.dma_start(out=e8, in_=e8_d[:, :, :, :])
        xsb = const.tile([P, T, BC], FP16)
        nc.sync.dma_start(out=xsb, in_=xT_d[:, :, :])
        # Gx: x-MLP outputs + combined L3 bias, [feat, gate, t*BC]
        gx = const.tile([P, 4, T * BC], FP16)

        # ---------------- x-path: 4 x-MLPs over all (t,b), chunks of 512 cols
        CH = 512  # moving-dim chunk (max fp32-psum free dim)
        TCH = CH // BC  # timesteps per chunk
        nchunks = (T * BC) // CH
        with (
            tc.tile_pool(name="xps", bufs=6, space="PSUM") as xps,
            tc.tile_pool(name="xact", bufs=3) as xact,
        ):
            for c in range(nchunks):
                rhs_x = xsb[:, c * TCH : (c + 1) * TCH, :]
                for g in range(4):
                    base = g * MLP_COLS
                    y1 = xact.tile([P, 2, CH], FP16, tag="y1")
                    for m in range(2):
                        ps = xps.tile([P, CH], F32, tag="xps")
                        nc.tensor.matmul(
                            ps,
                            wxs[:, base + m * P : base + (m + 1) * P],
                            rhs_x,
                            start=True,
                            stop=True,
                        )
                        s = g * 2 + m
                        # bias + relu fused on ScalarE (bias is per-partition here)
                        nc.scalar.activation(
                            y1[:, m, :], ps, AF.Relu, bias=bx1[:, s : s + 1]
                        )
                    y2 = xact.tile([P, 2, CH], FP16, tag="y2")
                    for m in range(2):
                        ps = xps.tile([P, CH], F32, tag="xps")
                        for k in range(2):
                            nc.tensor.matmul(
                                ps,
                                wxs[
                                    :,
                                    base + 256 + k * 256 + m * P : base
                                    + 256
                                    + k * 256
                                    + (m + 1) * P,
                                ],
                                y1[:, k, :],
                                start=(k == 0),
                                stop=(k == 1),
                            )
                        s = g * 2 + m
                        # bias + relu fused on VectorE (balance engines)
                        nc.vector.tensor_scalar(
                            out=y2[:, m, :],
                            in0=ps,
                            scalar1=bx2[:, s : s + 1],
                            scalar2=0.0,
                            op0=ALU.add,
                            op1=ALU.max,
                        )
                    ps3 = xps.tile([P, CH], F32, tag="xps")
                    for k in range(2):
                        nc.tensor.matmul(
                            ps3,
                            wxs[:, base + 768 + k * P : base + 768 + (k + 1) * P],
                            y2[:, k, :],
                            start=(k == 0),
                            stop=(k == 1),
                        )
                    # add combined bias (bx3 + bh3), store fp16 into Gx.
                    # (DVE, not ACT Identity: Identity lives in a different
                    # ACT table set than Sigmoid/Tanh and caused a ~1.3us
                    # ACT_TABLE_LOAD every recurrence loop iteration)
                    nc.vector.tensor_scalar_add(
                        gx[:, g, c * CH : (c + 1) * CH], ps3, bx3[:, g : g + 1]
                    )

        # ---------------- recurrence: two interleaved half-batch streams
        # Stream s covers batch cols [s*HB, (s+1)*HB); their per-step
        # dependency chains are independent, so stream B's matmuls overlap
        # stream A's elementwise/activation tail (and vice versa).
        HB = BC // 2
        state = ctx.enter_context(tc.tile_pool(name="state", bufs=1))
        hS = [state.tile([P, HB], FP16, tag=f"h{s}") for s in range(2)]
        cS = [state.tile([P, HB], F32, tag=f"c{s}") for s in range(2)]
        for s in range(2):
            nc.vector.memset(hS[s], 0.0)
            nc.vector.memset(cS[s], 0.0)

        with (
            tc.tile_pool(name="hps", bufs=1, space="PSUM") as hps,
            tc.tile_pool(name="hact", bufs=3) as hact,
        ):
            # persistent per-stream PSUM tiles (6 banks)
            ps1S = [hps.tile([P, 8, HB], F32, tag=f"ps1{s}") for s in range(2)]
            ps2S = [hps.tile([P, 8, HB], F32, tag=f"ps2{s}") for s in range(2)]
            ps3S = [hps.tile([P, 4, HB], F32, tag=f"ps3{s}") for s in range(2)]

            def step(s, gx_in):
                hT, cT = hS[s], cS[s]
                ps1, ps2, ps3 = ps1S[s], ps2S[s], ps3S[s]
                e8s = e8[:, s, :, :]
                # L1: bias seed (rank-8 one-hot) + 8 weight matmuls
                nc.tensor.matmul(ps1, bpk1, e8s, start=True, stop=True)
                for g in range(4):
                    base = g * MLP_COLS
                    for m in range(2):
                        nc.tensor.matmul(
                            ps1[:, g * 2 + m, :],
                            whs[:, base + m * P : base + (m + 1) * P],
                            hT,
                            start=False,
                            stop=True,
                            skip_group_check=True,
                        )
                y1 = hact.tile([P, 8, HB], FP16, tag=f"hy1{s}")
                nc.vector.tensor_scalar_max(y1, ps1, 0.0)

                nc.tensor.matmul(ps2, bpk2, e8s, start=True, stop=True)
                for g in range(4):
                    base = g * MLP_COLS
                    for m in range(2):
                        for k in range(2):
                            nc.tensor.matmul(
                                ps2[:, g * 2 + m, :],
                                whs[
                                    :,
                                    base + 256 + k * 256 + m * P : base
                                    + 256
                                    + k * 256
                                    + (m + 1) * P,
                                ],
                                y1[:, g * 2 + k, :],
                                start=False,
                                stop=(k == 1),
                                skip_group_check=True,
                            )
                y2 = hact.tile([P, 8, HB], FP16, tag=f"hy2{s}")
                nc.vector.tensor_scalar_max(y2, ps2, 0.0)

                for g in range(4):
                    base = g * MLP_COLS
                    for k in range(2):
                        nc.tensor.matmul(
                            ps3[:, g, :],
                            whs[:, base + 768 + k * P : base + 768 + (k + 1) * P],
                            y2[:, g * 2 + k, :],
                            start=(k == 0),
                            stop=(k == 1),
                        )
                pre = hact.tile([P, 4, HB], FP16, tag=f"pre{s}")
                nc.vector.tensor_tensor(out=pre, in0=ps3, in1=gx_in, op=ALU.add)
                gsig = hact.tile([P, 3, HB], FP16, tag=f"gsig{s}")
                nc.scalar.activation(gsig, pre[:, 0:3, :], AF.Sigmoid)
                gch = hact.tile([P, HB], FP16, tag=f"gch{s}")
                nc.scalar.activation(gch, pre[:, 3, :], AF.Tanh)
                fc = hact.tile([P, HB], F32, tag=f"fc{s}")
                nc.gpsimd.tensor_tensor(out=fc, in0=gsig[:, 0, :], in1=cT, op=ALU.mult)
                ic = hact.tile([P, HB], F32, tag=f"ic{s}")
                nc.gpsimd.tensor_tensor(
                    out=ic, in0=gsig[:, 1, :], in1=gch, op=ALU.mult
                )
                nc.vector.tensor_tensor(out=cT, in0=fc, in1=ic, op=ALU.add)
                th = hact.tile([P, HB], FP16, tag=f"th{s}")
                nc.scalar.activation(th, cT, AF.Tanh)
                nc.vector.tensor_tensor(out=hT, in0=gsig[:, 2, :], in1=th, op=ALU.mult)

            def gx_slice(src, col0):
                return src[:, :, col0 : col0 + HB]

            if unroll >= T:
                for t in range(T):
                    for s in range(2):
                        step(s, gx_slice(gx, t * BC + s * HB))
            else:
                assert T % unroll == 0
                with tc.For_i(
                    0, T, unroll, hint_engines=(mybir.EngineType.PE,)
                ) as iv:
                    # one dynamic-AP instruction per body: stage the Gx
                    # window for all `unroll` steps (static slices after);
                    # on GpSimd to keep it off the busy VectorE queue.  The
                    # offset register must live on the consuming engine.
                    t0 = nc.gpsimd.snap(
                        iv * BC, min_val=0, max_val=(T - unroll) * BC
                    )
                    stage = hact.tile([P, 4, unroll * BC], FP16, tag="gxstage")
                    nc.gpsimd.tensor_copy(stage, gx[:, :, ds(t0, unroll * BC)])
                    for j in range(unroll):
                        for s in range(2):
                            step(s, gx_slice(stage, j * BC + s * HB))

        outsb = state.tile([P, BC], F32)
        nc.vector.tensor_copy(outsb[:, 0:HB], hS[0])
        nc.vector.tensor_copy(outsb[:, HB:BC], hS[1])
        nc.sync.dma_start(out=out_d[:, :], in_=outsb)

    if compile:
        nc.compile()
    return nc


def _np(a):
    return np.asarray(a, dtype=np.float32)


def _prep_shared(params):
    """Pack weights/biases into the kernel's DRAM layouts (replicated)."""
    import ml_dtypes

    order = ["F_h", "I_h", "O_h", "C_hat_h", "F_x", "I_x", "O_x", "C_hat_x"]
    wpack = np.zeros((P, WPACK_COLS), dtype=np.float32)
    for i, name in enumerate(order):
        (W1, b1), (W2, b2), (W3, b3) = [( _np(w), _np(b)) for w, b in params[name]]
        base = i * MLP_COLS
        wpack[:, base : base + 256] = W1
        wpack[:, base + 256 : base + 512] = W2[0:128, :]
        wpack[:, base + 512 : base + 768] = W2[128:256, :]
        wpack[:, base + 768 : base + 896] = W3[0:128, :]
        wpack[:, base + 896 : base + 1024] = W3[128:256, :]
    whp = wpack[:, : 4 * MLP_COLS].astype(np.float16)
    wxp = wpack[:, 4 * MLP_COLS :].astype(np.float16)

    bpk1 = np.zeros((8, P), dtype=np.float16)
    bpk2 = np.zeros((8, P), dtype=np.float16)
    bx1 = np.zeros((P, 8), dtype=np.float32)
    bx2 = np.zeros((P, 8), dtype=np.float32)
    bx3 = np.zeros((P, 4), dtype=np.float32)
    for g, gate in enumerate(GATE_NAMES):
        bh = [_np(b) for _, b in params[gate + "_h"]]
        bx = [_np(b) for _, b in params[gate + "_x"]]
        for m in range(2):
            bpk1[g * 2 + m, :] = bh[0][m * P : (m + 1) * P].astype(np.float16)
            bpk2[g * 2 + m, :] = bh[1][m * P : (m + 1) * P].astype(np.float16)
            bx1[:, g * 2 + m] = bx[0][m * P : (m + 1) * P]
            bx2[:, g * 2 + m] = bx[1][m * P : (m + 1) * P]
        bx3[:, g] = bx[2] + bh[2]
    e8 = np.zeros((8, 2, 8, BC // 2), dtype=np.float16)
    for s in range(8):
        e8[s, :, s, :] = 1.0
    return dict(
        whp=whp, wxp=wxp, bpk1=bpk1, bpk2=bpk2, e8=e8, bx1=bx1, bx2=bx2, bx3=bx3
    )


def _run(x, params, T=SEQ, unroll=32, trace=False):
    x = _np(x)
    B = x.shape[0]
    assert B == BATCH and x.shape[2] == INPUT
    shared = _prep_shared(params)
    in_maps = []
    for core in range(NCORES):
        xc = x[core * BC : (core + 1) * BC, :T, :]  # [BC, T, 128]
        xT = np.ascontiguousarray(xc.transpose(2, 1, 0)).astype(np.float16)
        m = dict(shared)
        m["xT"] = xT
        in_maps.append(m)
    nc = build_nc(T=T, unroll=unroll)
    res = run_bass_kernel_spmd(nc, in_maps, core_ids=list(range(NCORES)), trace=trace)
    out = np.zeros((BATCH, HIDDEN), dtype=np.float32)
    for core in range(NCORES):
        out[core * BC : (core + 1) * BC, :] = res.results[core]["hT_out"].T
    return out, res


def kernel(x, params):
    out, _ = _run(x, params)
    return out


# revision 38
# speedup vs baseline: 1.0666x; 1.0296x over previous
"""DeepLSTM Trainium2 kernel (8 NeuronCores, data-parallel over batch).

Model (see reference): per timestep t:
    F = sigmoid(MLP_Fh(h) + MLP_Fx(x_t)); I,O likewise; C = tanh(MLP_Ch(h) + MLP_Cx(x_t))
    c = F*c + I*C ; h = O*tanh(c)
Each MLP is 128 -> 256 -> 256 -> 128 with ReLU between layers.

Strategy per core (batch shard of 32 rows):
  * The 4 x-MLPs do not depend on the recurrence: precompute them for all
    (t, b) as large feature-major matmuls (N=512 moving dim), results ("Gx",
    fp16, L3-bias folded in) stay resident in SBUF.
  * The 4 h-MLPs run in the sequential 512-step loop, fully in feature-major
    layout [feat=128 partitions, batch=32 free] so no transposes are needed
    anywhere.  Weights stationary fp16 (FWL), activations moving.
  * fp32 cell state + PSUM accumulation; fp16 weights/activations/Gx.

Host-side prep (not in HW exec time): shard batch, transpose x to
[feat, t, b] fp16, pack weights into one [128, 8192] fp16 tensor of
128-row k-slices, broadcast/fold biases.
"""

import os

os.environ.setdefault("MYCRO_LOCAL_CACHE", "1")

from contextlib import ExitStack

import numpy as np

import concourse.bass as bass
import concourse.mybir as mybir
import concourse.tile as tile
from concourse import bacc
from concourse.bass import ds
from concourse.bass_utils import run_bass_kernel_spmd

P = 128
HIDDEN = 128
INPUT = 128
BATCH = 256
SEQ = 512
NCORES = 8
BC = BATCH // NCORES  # 32 batch rows per core
GATE_NAMES = ["F", "I", "O", "C_hat"]  # F,I,O sigmoid; C_hat tanh
FP16 = mybir.dt.float16
F32 = mybir.dt.float32
AF = mybir.ActivationFunctionType
ALU = mybir.AluOpType

# weight pack layout: 8 MLPs (F_h,I_h,O_h,C_hat_h,F_x,I_x,O_x,C_hat_x),
# 1024 columns each:
#   [0,256)    W1 [128in, 256out]
#   [256,512)  W2 rows 0:128   (k-slice 0) [128, 256]
#   [512,768)  W2 rows 128:256 (k-slice 1) [128, 256]
#   [768,896)  W3 rows 0:128   [128, 128]
#   [896,1024) W3 rows 128:256 [128, 128]
MLP_COLS = 1024
WPACK_COLS = 8 * MLP_COLS


def build_nc(T=SEQ, unroll=32, compile=True):
    nc = bacc.Bacc()
    FP8 = mybir.dt.float8e4
    xT_d = nc.dram_tensor("xT", [P, T, BC], FP16, kind="ExternalInput")
    # h-MLP weights in fp8e4m3: FWL loads 4 cols/cycle (vs 2 for fp16),
    # halving the LDWEIGHTS stream that dominates the recurrence PE time.
    whp_d = nc.dram_tensor("whp", [P, 4 * MLP_COLS], FP16, kind="ExternalInput")
    wxp_d = nc.dram_tensor("wxp", [P, 4 * MLP_COLS], FP16, kind="ExternalInput")
    bx1_d = nc.dram_tensor("bx1", [P, 8], F32, kind="ExternalInput")
    bx2_d = nc.dram_tensor("bx2", [P, 8], F32, kind="ExternalInput")
    bx3_d = nc.dram_tensor("bx3", [P, 4], F32, kind="ExternalInput")
    # rank-8 bias injection: bpk{1,2}[s, m] = layer bias of slot s (=g*2+m),
    # e8[k, (s, b)] = (k == s); psum += bpk.T @ e8 broadcasts each slot's
    # per-feature bias across its batch columns in one matmul.
    bpk1_d = nc.dram_tensor("bpk1", [8, P], FP16, kind="ExternalInput")
    bpk2_d = nc.dram_tensor("bpk2", [8, P], FP16, kind="ExternalInput")
    e8_d = nc.dram_tensor("e8", [8, 2, 8, BC // 2], FP16, kind="ExternalInput")
    out_d = nc.dram_tensor("hT_out", [P, BC], F32, kind="ExternalOutput")

    with tile.TileContext(nc) as tc, ExitStack() as ctx:
        const = ctx.enter_context(tc.tile_pool(name="const", bufs=1))
        whs = const.tile([P, 4 * MLP_COLS], FP16)
        nc.sync.dma_start(out=whs, in_=whp_d[:, :])
        wxs = const.tile([P, 4 * MLP_COLS], FP16)
        nc.sync.dma_start(out=wxs, in_=wxp_d[:, :])
        bx1 = const.tile([P, 8], F32)
        nc.sync.dma_start(out=bx1, in_=bx1_d[:, :])
        bx2 = const.tile([P, 8], F32)
        nc.sync.dma_start(out=bx2, in_=bx2_d[:, :])
        bx3 = const.tile([P, 4], F32)
        nc.sync.dma_start(out=bx3, in_=bx3_d[:, :])
        bpk1 = const.tile([8, P], FP16)
        nc.sync.dma_start(out=bpk1, in_=bpk1_d[:, :])
        bpk2 = const.tile([8, P], FP16)
        nc.sync.dma_start(out=bpk2, in_=bpk2_d[:, :])
        e8 = const.tile([8, 2, 8, BC // 2], FP16)
        nc.sync.dma_start(out=e8, in_=e8_d[:, :, :, :])
        xsb = const.tile([P, T, BC], FP16)
        nc.sync.dma_start(out=xsb, in_=xT_d[:, :, :])
        # Gx: x-MLP outputs + combined L3 bias, [feat, gate, t*BC]
        gx = const.tile([P, 4, T * BC], FP16)

        # ---------------- x-path: 4 x-MLPs over all (t,b), chunks of 512 cols
        CH = 512  # moving-dim chunk (max fp32-psum free dim)
        TCH = CH // BC  # timesteps per chunk
        nchunks = (T * BC) // CH
        with (
            tc.tile_pool(name="xps", bufs=6, space="PSUM") as xps,
            tc.tile_pool(name="xact", bufs=3) as xact,
        ):
            for c in range(nchunks):
                rhs_x = xsb[:, c * TCH : (c + 1) * TCH, :]
                for g in range(4):
                    base = g * MLP_COLS
                    y1 = xact.tile([P, 2, CH], FP16, tag="y1")
                    for m in range(2):
                        ps = xps.tile([P, CH], F32, tag="xps")
                        nc.tensor.matmul(
                            ps,
                            wxs[:, base + m * P : base + (m + 1) * P],
                            rhs_x,
                            start=True,
                            stop=True,
                        )
                        s = g * 2 + m
                        # bias + relu fused on ScalarE (bias is per-partition here)
                        nc.scalar.activation(
                            y1[:, m, :], ps, AF.Relu, bias=bx1[:, s : s + 1]
                        )
                    y2 = xact.tile([P, 2, CH], FP16, tag="y2")
                    for m in range(2):
                        ps = xps.tile([P, CH], F32, tag="xps")
                        for k in range(2):
                            nc.tensor.matmul(
                                ps,
                                wxs[
                                    :,
                                    base + 256 + k * 256 + m * P : base
                                    + 256
                                    + k * 256
                                    + (m + 1) * P,
                                ],
                                y1[:, k, :],
                                start=(k == 0),
                                stop=(k == 1),
                            )
                        s = g * 2 + m
                        # bias + relu fused on VectorE (balance engines)
                        nc.vector.tensor_scalar(
                            out=y2[:, m, :],
                            in0=ps,
                            scalar1=bx2[:, s : s + 1],
                            scalar2=0.0,
                            op0=ALU.add,
                            op1=ALU.max,
                        )
                    ps3 = xps.tile([P, CH], F32, tag="xps")
                    for k in range(2):
                        nc.tensor.matmul(
                            ps3,
                            wxs[:, base + 768 + k * P : base + 768 + (k + 1) * P],
                            y2[:, k, :],
                            start=(k == 0),
                            stop=(k == 1),
                        )
                    # add combined bias (bx3 + bh3), store fp16 into Gx.
                    # (DVE, not ACT Identity: Identity lives in a different
                    # ACT table set than Sigmoid/Tanh and caused a ~1.3us
                    # ACT_TABLE_LOAD every recurrence loop iteration)
                    nc.vector.tensor_scalar_add(
                        gx[:, g, c * CH : (c + 1) * CH], ps3, bx3[:, g : g + 1]
                    )

        # ---------------- recurrence: two interleaved half-batch streams
        # Stream s covers batch cols [s*HB, (s+1)*HB); their per-step
        # dependency chains are independent, so stream B's matmuls overlap
        # stream A's elementwise/activation tail (and vice versa).
        HB = BC // 2
        state = ctx.enter_context(tc.tile_pool(name="state", bufs=1))
        hS = [state.tile([P, HB], FP16, tag=f"h{s}") for s in range(2)]
        cS = [state.tile([P, HB], F32, tag=f"c{s}") for s in range(2)]
        for s in range(2):
            nc.vector.memset(hS[s], 0.0)
            nc.vector.memset(cS[s], 0.0)

        with (
            tc.tile_pool(name="hps", bufs=1, space="PSUM") as hps,
            tc.tile_pool(name="hact", bufs=3) as hact,
        ):
            # persistent per-stream PSUM tiles (6 banks)
            ps1S = [hps.tile([P, 8, HB], F32, tag=f"ps1{s}") for s in range(2)]
            ps2S = [hps.tile([P, 8, HB], F32, tag=f"ps2{s}") for s in range(2)]
            ps3S = [hps.tile([P, 4, HB], F32, tag=f"ps3{s}") for s in range(2)]

            def step(s, gx_in):
                hT, cT = hS[s], cS[s]
                ps1, ps2, ps3 = ps1S[s], ps2S[s], ps3S[s]
                e8s = e8[:, s, :, :]
                # L1: bias seed (rank-8 one-hot) + 8 weight matmuls
                nc.tensor.matmul(ps1, bpk1, e8s, start=True, stop=True)
                for g in range(4):
                    base = g * MLP_COLS
                    for m in range(2):
                        nc.tensor.matmul(
                            ps1[:, g * 2 + m, :],
                            whs[:, base + m * P : base + (m + 1) * P],
                            hT,
                            start=False,
                            stop=True,
                            skip_group_check=True,
                        )
                y1 = hact.tile([P, 8, HB], FP16, tag=f"hy1{s}")
                nc.vector.tensor_scalar_max(y1, ps1, 0.0)

                nc.tensor.matmul(ps2, bpk2, e8s, start=True, stop=True)
                for g in range(4):
                    base = g * MLP_COLS
                    for m in range(2):
                        for k in range(2):
                            nc.tensor.matmul(
                                ps2[:, g * 2 + m, :],
                                whs[
                                    :,
                                    base + 256 + k * 256 + m * P : base
                                    + 256
                                    + k * 256
                                    + (m + 1) * P,
                                ],
                                y1[:, g * 2 + k, :],
                                start=False,
                                stop=(k == 1),
                                skip_group_check=True,
                            )
                y2 = hact.tile([P, 8, HB], FP16, tag=f"hy2{s}")
                nc.vector.tensor_scalar_max(y2, ps2, 0.0)

                for g in range(4):
                    base = g * MLP_COLS
                    for k in range(2):
                        nc.tensor.matmul(
                            ps3[:, g, :],
                            whs[:, base + 768 + k * P : base + 768 + (k + 1) * P],
                            y2[:, g * 2 + k, :],
                            start=(k == 0),
                            stop=(k == 1),
                        )
                pre = hact.tile([P, 4, HB], FP16, tag=f"pre{s}")
                nc.vector.tensor_tensor(out=pre, in0=ps3, in1=gx_in, op=ALU.add)
                gsig = hact.tile([P, 3, HB], FP16, tag=f"gsig{s}")
                nc.scalar.activation(gsig, pre[:, 0:3, :], AF.Sigmoid)
                gch = hact.tile([P, HB], FP16, tag=f"gch{s}")
                nc.scalar.activation(gch, pre[:, 3, :], AF.Tanh)
                fc = hact.tile([P, HB], F32, tag=f"fc{s}")
                nc.gpsimd.tensor_tensor(out=fc, in0=gsig[:, 0, :], in1=cT, op=ALU.mult)
                ic = hact.tile([P, HB], F32, tag=f"ic{s}")
                nc.gpsimd.tensor_tensor(
                    out=ic, in0=gsig[:, 1, :], in1=gch, op=ALU.mult
                )
                nc.vector.tensor_tensor(out=cT, in0=fc, in1=ic, op=ALU.add)
                th = hact.tile([P, HB], FP16, tag=f"th{s}")
                nc.scalar.activation(th, cT, AF.Tanh)
                nc.vector.tensor_tensor(out=hT, in0=gsig[:, 2, :], in1=th, op=ALU.mult)

            def gx_slice(src, col0):
                return src[:, :, col0 : col0 + HB]

            if unroll >= T:
                for t in range(T):
                    for s in range(2):
                        step(s, gx_slice(gx, t * BC + s * HB))
            else:
                assert T % unroll == 0
                with tc.For_i(
                    0, T, unroll, hint_engines=(mybir.EngineType.PE,)
                ) as iv:
                    # one dynamic-AP instruction per body: stage the Gx
                    # window for all `unroll` steps (static slices after);
                    # on GpSimd to keep it off the busy VectorE queue.  The
                    # offset register must live on the consuming engine.
                    t0 = nc.gpsimd.snap(
                        iv * BC, min_val=0, max_val=(T - unroll) * BC
                    )
                    stage = hact.tile([P, 4, unroll * BC], FP16, tag="gxstage")
                    nc.gpsimd.tensor_copy(stage, gx[:, :, ds(t0, unroll * BC)])
                    for j in range(unroll):
                        for s in range(2):
                            step(s, gx_slice(stage, j * BC + s * HB))

        outsb = state.tile([P, BC], F32)
        nc.vector.tensor_copy(outsb[:, 0:HB], hS[0])
        nc.vector.tensor_copy(outsb[:, HB:BC], hS[1])
        nc.sync.dma_start(out=out_d[:, :], in_=outsb)

    if compile:
        nc.compile()
    return nc


def _np(a):
    return np.asarray(a, dtype=np.float32)


def _prep_shared(params):
    """Pack weights/biases into the kernel's DRAM layouts (replicated)."""
    import ml_dtypes

    order = ["F_h", "I_h", "O_h", "C_hat_h", "F_x", "I_x", "O_x", "C_hat_x"]
    wpack = np.zeros((P, WPACK_COLS), dtype=np.float32)
    for i, name in enumerate(order):
        (W1, b1), (W2, b2), (W3, b3) = [( _np(w), _np(b)) for w, b in params[name]]
        base = i * MLP_COLS
        wpack[:, base : base + 256] = W1
        wpack[:, base + 256 : base + 512] = W2[0:128, :]
        wpack[:, base + 512 : base + 768] = W2[128:256, :]
        wpack[:, base + 768 : base + 896] = W3[0:128, :]
        wpack[:, base + 896 : base + 1024] = W3[128:256, :]
    whp = wpack[:, : 4 * MLP_COLS].astype(np.float16)
    wxp = wpack[:, 4 * MLP_COLS :].astype(np.float16)

    bpk1 = np.zeros((8, P), dtype=np.float16)
    bpk2 = np.zeros((8, P), dtype=np.float16)
    bx1 = np.zeros((P, 8), dtype=np.float32)
    bx2 = np.zeros((P, 8), dtype=np.float32)
    bx3 = np.zeros((P, 4), dtype=np.float32)
    for g, gate in enumerate(GATE_NAMES):
        bh = [_np(b) for _, b in params[gate + "_h"]]
        bx = [_np(b) for _, b in params[gate + "_x"]]
        for m in range(2):
            bpk1[g * 2 + m, :] = bh[0][m * P : (m + 1) * P].astype(np.float16)
            bpk2[g * 2 + m, :] = bh[1][m * P : (m + 1) * P].astype(np.float16)
            bx1[:, g * 2 + m] = bx[0][m * P : (m + 1) * P]
            bx2[:, g * 2 + m] = bx[1][m * P : (m + 1) * P]
        bx3[:, g] = bx[2] + bh[2]
    e8 = np.zeros((8, 2, 8, BC // 2), dtype=np.float16)
    for s in range(8):
        e8[s, :, s, :] = 1.0
    return dict(
        whp=whp, wxp=wxp, bpk1=bpk1, bpk2=bpk2, e8=e8, bx1=bx1, bx2=bx2, bx3=bx3
    )


def _run(x, params, T=SEQ, unroll=32, trace=False):
    x = _np(x)
    B = x.shape[0]
    assert B == BATCH and x.shape[2] == INPUT
    shared = _prep_shared(params)
    in_maps = []
    for core in range(NCORES):
        xc = x[core * BC : (core + 1) * BC, :T, :]  # [BC, T, 128]
        xT = np.ascontiguousarray(xc.transpose(2, 1, 0)).astype(np.float16)
        m = dict(shared)
        m["xT"] = xT
        in_maps.append(m)
    nc = build_nc(T=T, unroll=unroll)
    res = run_bass_kernel_spmd(nc, in_maps, core_ids=list(range(NCORES)), trace=trace)
    out = np.zeros((BATCH, HIDDEN), dtype=np.float32)
    for core in range(NCORES):
        out[core * BC : (core + 1) * BC, :] = res.results[core]["hT_out"].T
    return out, res


def kernel(x, params):
    out, _ = _run(x, params)
    return out
